# revision 18
# baseline (speedup 1.0000x reference)
"""Trainium2 Bass kernel for nn_CoevolutionAnalyzer (pairwise-MLP coevolution scores).

Math (per batch q):
    g = domain * evo                         [512, 128]
    a = g @ W1[:128], c = g @ W1[128:]       [512, 128]
    h_ij  = relu(a_i + c_j + b1)             [128]
    z2    = W2.T h_ij + b2 ; h2 = relu(z2)   [64]
    s_ij  = sigmoid(W3.h2 + b3)
    out   = triu(s,1) + triu(s,1).T

Sharding (8 cores, one SPMD program):
    Only j >= 64*floor(i/64) is computed (upper triangle padded to the 64-column
    block grid; the pad is discarded on the host via triu). Every core takes 8
    rows of every 64-row block (rows 64*b + 8*k + m) for both batches, so all
    cores run the identical instruction stream; only DMA'd data differs. The
    i-side inputs are host-gathered per core so on-device column indices are
    core-independent.

Per block (8 rows = 4 pairs, j-window [64b, 512) of length L):
    stage1 (DVE/GpSimd): h = relu(cT + a_i + b1) as bf16, two stacked tiles
            (even rows / odd rows), G pairs side by side (G*L <= 512)
    stage2 (PE bf16):   W2.T @ h_even -> psum[0:64], W2.T @ h_odd -> [64:128]
    relu2  (ACT):       h2 = relu(psum + [b2;b2]) -> bf16
    stage3 (PE bf16):   one matmul per round with a replicated-W3 stationary
            [128, 128//R]; pair u's scores land at psum partitions 32u+{0,1}
    sigmoid(ACT):       into a per-batch staging tile [128, 3328]
    out:                2 strided-partition DMAs per batch (rows 32a / 32a+1)
"""

import os

import numpy as np
from ml_dtypes import bfloat16 as bf16_np

import concourse.bass as bass
import concourse.tile as tile
from concourse import bacc, mybir
from concourse.bass_utils import run_bass_kernel_spmd

B = 2
N = 512
D = 128
NB = 8          # number of 64-row j-blocks
BLK = N // NB   # 64
RPB = 8         # rows per core per block
NI = NB * RPB   # i-rows per core per batch (64)
F32 = mybir.dt.float32
F32R = mybir.dt.float32r
BF16 = mybir.dt.bfloat16
AF = mybir.ActivationFunctionType
ALU = mybir.AluOpType

# per-block geometry
LS = [N - BLK * b for b in range(NB)]            # j-window lengths
GS = [min(4, N // L) for L in LS]                # pairs per round
RS = [4 // g for g in GS]                        # rounds per block
WS = [GS[b] * LS[b] for b in range(NB)]          # sig segment widths
SIGW = sum(WS)                                   # 3328
SIGBASE = [sum(WS[:b]) for b in range(NB)]

# how many of the 8 stage-1 rows per block go to GpSimd instead of DVE
GPS_ROWS = 0

LAST_RESULT = None  # set by kernel(); test harness reads exec_time_ns


def _build():
    nc = bacc.Bacc("TRN2", target_bir_lowering=False, debug=False)

    # din columns: di(2*64) | ei(2*64) | dj(2*512) | ej(2*512)  => 2304 columns
    din = nc.declare_dram_parameter("din", [D, 2 * NI * B + 2 * N * B], BF16, isOutput=False)
    wf = nc.declare_dram_parameter("wf", [D, 2 * D], BF16, isOutput=False)    # w1a|w1b
    bb = nc.declare_dram_parameter("bb", [D, 3], F32, isOutput=False)         # b1|b2s|b3
    wb = nc.declare_dram_parameter("wb", [D, D // 2 + D], BF16, isOutput=False)  # w2|w3w
    out = nc.declare_dram_parameter("out", [B, 4, 2, SIGW], F32, isOutput=True)

    DI, EI, DJ, EJ = 0, NI * B, 2 * NI * B, 2 * NI * B + N * B

    with tile.TileContext(nc) as tc:
        with (
            tc.tile_pool(name="singles", bufs=1) as singles,
            tc.tile_pool(name="per_batch", bufs=2) as per_batch,
            tc.tile_pool(name="hpool", bufs=6) as hpool,
            tc.tile_pool(name="h2pool", bufs=3) as h2pool,
            tc.tile_pool(name="psz", bufs=2, space="PSUM") as psz_pool,
            tc.tile_pool(name="pss", bufs=2, space="PSUM") as pss_pool,
            tc.tile_pool(name="pset", bufs=1, space="PSUM") as pset_pool,
        ):
            s_in = singles.tile([D, 2 * NI * B + 2 * N * B], BF16)
            s_wf = singles.tile([D, 2 * D], BF16)
            s_bb = singles.tile([D, 3], F32)
            s_wb = singles.tile([D, D // 2 + D], BF16)
            # batch-0 j-side + weights first so compute can start early
            H = N // 2
            nc.scalar.dma_start(out=s_in[:, DJ : DJ + H], in_=din[:, DJ : DJ + H])
            nc.sync.dma_start(out=s_in[:, DJ + H : DJ + N], in_=din[:, DJ + H : DJ + N])
            nc.scalar.dma_start(out=s_in[:, EJ : EJ + H], in_=din[:, EJ : EJ + H])
            nc.sync.dma_start(out=s_in[:, EJ + H : EJ + N], in_=din[:, EJ + H : EJ + N])
            nc.scalar.dma_start(out=s_wf, in_=wf[:])
            nc.scalar.dma_start(out=s_bb, in_=bb[:])
            nc.scalar.dma_start(out=s_wb, in_=wb[:])
            nc.scalar.dma_start(out=s_in[:, : 2 * NI * B], in_=din[:, : 2 * NI * B])
            nc.sync.dma_start(
                out=s_in[:, DJ + N : DJ + 2 * N], in_=din[:, DJ + N : DJ + 2 * N]
            )
            nc.sync.dma_start(
                out=s_in[:, EJ + N : EJ + 2 * N], in_=din[:, EJ + N : EJ + 2 * N]
            )
            s_w2 = s_wb[:, : D // 2]
            s_w3w = s_wb[:, D // 2 :]
            s_b1 = s_bb[:, 0:1]
            s_b2s = s_bb[:, 1:2]
            s_b3 = s_bb[:, 2:3]

            for q in range(B):
                # --- per-batch setup: gT, aT(+b1), cT ---
                gti = per_batch.tile([D, NI], BF16, tag="gti")
                nc.vector.tensor_mul(
                    gti,
                    s_in[:, DI + q * NI : DI + (q + 1) * NI],
                    s_in[:, EI + q * NI : EI + (q + 1) * NI],
                )
                gtj = per_batch.tile([D, N], BF16, tag="gtj")
                nc.vector.tensor_mul(
                    gtj,
                    s_in[:, DJ + q * N : DJ + (q + 1) * N],
                    s_in[:, EJ + q * N : EJ + (q + 1) * N],
                )
                ps_a = pset_pool.tile([D, NI], F32, tag="ps_a")
                nc.tensor.matmul(ps_a[:], s_wf[:, :D], gti[:])
                ps_c = pset_pool.tile([D, N], F32, tag="ps_c")
                nc.tensor.matmul(ps_c[:], s_wf[:, D:], gtj[:])
                abt = per_batch.tile([D, NI], F32, tag="abt")
                nc.vector.tensor_scalar_add(abt, ps_a[:], s_b1)
                ct = per_batch.tile([D, N], BF16, tag="ct")
                nc.scalar.copy(ct, ps_c[:])

                sig = per_batch.tile([D, SIGW], F32, tag="sig")

                # --- j-block loop ---
                for b in range(NB):
                    j0 = BLK * b
                    L, G, R, W = LS[b], GS[b], RS[b], WS[b]
                    M = 128 // R
                    pss = pss_pool.tile([D, N], F32, tag="pss")
                    for rr in range((R + 1) // 2):
                        nrounds = min(2, R - 2 * rr)
                        psz = psz_pool.tile([D, 2 * N], F32, tag="psz")
                        h2 = h2pool.tile([D, 2 * N], BF16, tag="h2")
                        for rh in range(nrounds):
                            r = 2 * rr + rh
                            hs0 = hpool.tile([D, N], BF16, tag="hs0")
                            hs1 = hpool.tile([D, N], BF16, tag="hs1")
                            for g in range(G):
                                u = r * G + g  # pair index within block
                                for half in range(2):
                                    m = 2 * u + half  # local row 0..7
                                    hs = hs1 if half else hs0
                                    nc.vector.tensor_scalar(
                                        out=hs[:, g * L : (g + 1) * L],
                                        in0=ct[:, j0:N],
                                        scalar1=abt[:, b * RPB + m : b * RPB + m + 1],
                                        scalar2=0.0,
                                        op0=ALU.add,
                                        op1=ALU.max,
                                    )
                            nc.tensor.matmul(
                                psz[0 : D // 2, rh * N : rh * N + W],
                                s_w2,
                                hs0[:, :W],
                                tile_position=(0, 0),
                            )
                            nc.tensor.matmul(
                                psz[D // 2 : D, rh * N : rh * N + W],
                                s_w2,
                                hs1[:, :W],
                                tile_position=(0, 64),
                            )
                        if nrounds == 2:
                            nc.scalar.activation(
                                out=h2[:, : 2 * W].rearrange(
                                    "p (s w) -> p s w", s=2
                                ),
                                in_=psz[:, :].rearrange("p (s w) -> p s w", s=2)[
                                    :, :, :W
                                ],
                                func=AF.Relu,
                                bias=s_b2s,
                            )
                        else:
                            nc.scalar.activation(
                                out=h2[:, :W],
                                in_=psz[:, :W],
                                func=AF.Relu,
                                bias=s_b2s,
                            )
                        for rh in range(nrounds):
                            r = 2 * rr + rh
                            nc.tensor.matmul(
                                pss[M * r : M * (r + 1), :W],
                                s_w3w[:, :M],
                                h2[:, rh * W : rh * W + W],
                                tile_position=(0, M * r),
                            )
                    cb = SIGBASE[b]
                    nc.scalar.activation(
                        out=sig[:, cb : cb + W],
                        in_=pss[:, :W],
                        func=AF.Sigmoid,
                        bias=s_b3,
                    )
                # --- 4 output DMAs for the whole batch (contiguous rows) ---
                for a in range(4):
                    nc.sync.dma_start(
                        out=out[q, a], in_=sig[32 * a : 32 * a + 2, :]
                    )

    nc.compile()
    return nc


def build_in_maps(dom, evo, W1, b1, W2, b2, W3, b3):
    w3w = np.zeros((D, D), np.float32)
    w3w[: D // 2, 0::32] = W3[:, 0:1].repeat(4, axis=1)
    w3w[D // 2 :, 1::32] = W3[:, 0:1].repeat(4, axis=1)
    wb = np.concatenate([W2, w3w], axis=1).astype(bf16_np)
    wf = np.ascontiguousarray(np.concatenate([W1[:D], W1[D:]], axis=1)).astype(bf16_np)
    bbt = np.zeros((D, 3), np.float32)
    bbt[:, 0] = b1
    bbt[:, 1] = np.concatenate([b2, b2])
    bbt[:, 2] = float(b3[0])

    dom_jT = np.concatenate([dom[q].T for q in range(B)], axis=1)
    evo_jT = np.concatenate([evo[q].T for q in range(B)], axis=1)

    in_maps = []
    for k in range(8):
        rows = np.concatenate(
            [BLK * bb_ + RPB * k + np.arange(RPB) for bb_ in range(NB)]
        )
        dom_iT = np.concatenate([dom[q][rows].T for q in range(B)], axis=1)
        evo_iT = np.concatenate([evo[q][rows].T for q in range(B)], axis=1)
        din = np.ascontiguousarray(
            np.concatenate([dom_iT, evo_iT, dom_jT, evo_jT], axis=1)
        ).astype(bf16_np)
        in_maps.append({"din": din, "wf": wf, "bb": bbt, "wb": wb})
    return in_maps


def unpack_results(results):
    S = np.zeros((B, N, N), np.float32)
    for k in range(8):
        o = results[k]["out"]  # [B, 4, 2, SIGW]
        for q in range(B):
            for b in range(NB):
                L, G, R, W = LS[b], GS[b], RS[b], WS[b]
                seg = o[q, :, :, SIGBASE[b] : SIGBASE[b] + W]  # [4, 2, W]
                for a in range(4):
                    for g in range(G):
                        u = (a // (4 // R)) * G + g
                        i = BLK * b + RPB * k + 2 * u
                        S[q, i, BLK * b : BLK * b + L] = seg[a, 0, g * L : (g + 1) * L]
                        S[q, i + 1, BLK * b : BLK * b + L] = seg[
                            a, 1, g * L : (g + 1) * L
                        ]
    upper = np.triu(S, 1)
    return (upper + upper.transpose(0, 2, 1)).astype(np.float32)


def kernel(
    domain_features,
    evolutionary_features,
    W1,
    b1,
    W2,
    b2,
    W3,
    b3,
):
    global LAST_RESULT
    dom = np.ascontiguousarray(np.asarray(domain_features, dtype=np.float32))
    evo = np.ascontiguousarray(np.asarray(evolutionary_features, dtype=np.float32))
    W1 = np.asarray(W1, dtype=np.float32)
    b1 = np.asarray(b1, dtype=np.float32)
    W2 = np.asarray(W2, dtype=np.float32)
    b2 = np.asarray(b2, dtype=np.float32)
    W3 = np.asarray(W3, dtype=np.float32)
    b3 = np.asarray(b3, dtype=np.float32)

    nc = _build()
    in_maps = build_in_maps(dom, evo, W1, b1, W2, b2, W3, b3)

    trace = os.environ.get("KERNEL_TRACE", "0") == "1"
    res = run_bass_kernel_spmd(nc, in_maps, core_ids=list(range(8)), trace=trace)
    LAST_RESULT = res

    return unpack_results(res.results)


# revision 19
# speedup vs baseline: 1.0219x; 1.0219x over previous
"""Trainium2 Bass kernel for nn_CoevolutionAnalyzer (pairwise-MLP coevolution scores).

Math (per batch q):
    g = domain * evo                         [512, 128]
    a = g @ W1[:128], c = g @ W1[128:]       [512, 128]
    h_ij  = relu(a_i + c_j + b1)             [128]
    z2    = W2.T h_ij + b2 ; h2 = relu(z2)   [64]
    s_ij  = sigmoid(W3.h2 + b3)
    out   = triu(s,1) + triu(s,1).T

Sharding (8 cores, one SPMD program):
    Only j >= 64*floor(i/64) is computed (upper triangle padded to the 64-column
    block grid; the pad is discarded on the host via triu). Every core takes 8
    rows of every 64-row block (rows 64*b + 8*k + m) for both batches, so all
    cores run the identical instruction stream; only DMA'd data differs. The
    i-side inputs are host-gathered per core so on-device column indices are
    core-independent.

Per block (8 rows = 4 pairs, j-window [64b, 512) of length L):
    stage1 (DVE/GpSimd): h = relu(cT + a_i + b1) as bf16, two stacked tiles
            (even rows / odd rows), G pairs side by side (G*L <= 512)
    stage2 (PE bf16):   W2.T @ h_even -> psum[0:64], W2.T @ h_odd -> [64:128]
    relu2  (ACT):       h2 = relu(psum + [b2;b2]) -> bf16
    stage3 (PE bf16):   one matmul per round with a replicated-W3 stationary
            [128, 128//R]; pair u's scores land at psum partitions 32u+{0,1}
    sigmoid(ACT):       into a per-batch staging tile [128, 3328]
    out:                2 strided-partition DMAs per batch (rows 32a / 32a+1)
"""

import os

import numpy as np
from ml_dtypes import bfloat16 as bf16_np

import concourse.bass as bass
import concourse.tile as tile
from concourse import bacc, mybir
from concourse.bass_utils import run_bass_kernel_spmd

B = 2
N = 512
D = 128
NB = 8          # number of 64-row j-blocks
BLK = N // NB   # 64
RPB = 8         # rows per core per block
NI = NB * RPB   # i-rows per core per batch (64)
F32 = mybir.dt.float32
F32R = mybir.dt.float32r
BF16 = mybir.dt.bfloat16
AF = mybir.ActivationFunctionType
ALU = mybir.AluOpType

# per-block geometry
LS = [N - BLK * b for b in range(NB)]            # j-window lengths
GS = [min(4, N // L) for L in LS]                # pairs per round
RS = [4 // g for g in GS]                        # rounds per block
WS = [GS[b] * LS[b] for b in range(NB)]          # sig segment widths
SIGW = sum(WS)                                   # 3328
SIGBASE = [sum(WS[:b]) for b in range(NB)]

# how many of the 8 stage-1 rows per block go to GpSimd instead of DVE
GPS_ROWS = 0

LAST_RESULT = None  # set by kernel(); test harness reads exec_time_ns


def _build():
    nc = bacc.Bacc("TRN2", target_bir_lowering=False, debug=False, num_devices=8)

    # din columns: di(2*64) | ei(2*64) | dj(2*512) | ej(2*512)  => 2304 columns
    din = nc.declare_dram_parameter("din", [D, 2 * NI * B + 2 * N * B], BF16, isOutput=False)
    wf = nc.declare_dram_parameter("wf", [D, 2 * D], BF16, isOutput=False)    # w1a|w1b
    bb = nc.declare_dram_parameter("bb", [D, 3], F32, isOutput=False)         # b1|b2s|b3
    wb = nc.declare_dram_parameter("wb", [D, D // 2 + D], BF16, isOutput=False)  # w2|w3w
    out = nc.declare_dram_parameter("out", [B, 4, 2, SIGW], F32, isOutput=True)

    DI, EI, DJ, EJ = 0, NI * B, 2 * NI * B, 2 * NI * B + N * B

    with tile.TileContext(nc) as tc:
        with (
            tc.tile_pool(name="singles", bufs=1) as singles,
            tc.tile_pool(name="per_batch", bufs=2) as per_batch,
            tc.tile_pool(name="hpool", bufs=6) as hpool,
            tc.tile_pool(name="h2pool", bufs=3) as h2pool,
            tc.tile_pool(name="psz", bufs=2, space="PSUM") as psz_pool,
            tc.tile_pool(name="pss", bufs=2, space="PSUM") as pss_pool,
            tc.tile_pool(name="pset", bufs=1, space="PSUM") as pset_pool,
        ):
            s_in = singles.tile([D, 2 * NI * B + 2 * N * B], BF16)
            s_wf = singles.tile([D, 2 * D], BF16)
            s_bb = singles.tile([D, 3], F32)
            s_wb = singles.tile([D, D // 2 + D], BF16)
            # batch-0 j-side + weights first so compute can start early
            H = N // 2
            nc.scalar.dma_start(out=s_in[:, DJ : DJ + H], in_=din[:, DJ : DJ + H])
            nc.sync.dma_start(out=s_in[:, DJ + H : DJ + N], in_=din[:, DJ + H : DJ + N])
            nc.scalar.dma_start(out=s_in[:, EJ : EJ + H], in_=din[:, EJ : EJ + H])
            nc.sync.dma_start(out=s_in[:, EJ + H : EJ + N], in_=din[:, EJ + H : EJ + N])
            nc.scalar.dma_start(out=s_wf, in_=wf[:])
            nc.scalar.dma_start(out=s_bb, in_=bb[:])
            nc.scalar.dma_start(out=s_wb, in_=wb[:])
            nc.scalar.dma_start(out=s_in[:, : 2 * NI * B], in_=din[:, : 2 * NI * B])
            nc.sync.dma_start(
                out=s_in[:, DJ + N : DJ + 2 * N], in_=din[:, DJ + N : DJ + 2 * N]
            )
            nc.sync.dma_start(
                out=s_in[:, EJ + N : EJ + 2 * N], in_=din[:, EJ + N : EJ + 2 * N]
            )
            s_w2 = s_wb[:, : D // 2]
            s_w3w = s_wb[:, D // 2 :]
            s_b1 = s_bb[:, 0:1]
            s_b2s = s_bb[:, 1:2]
            s_b3 = s_bb[:, 2:3]

            for q in range(B):
                # --- per-batch setup: gT, aT(+b1), cT ---
                gti = per_batch.tile([D, NI], BF16, tag="gti")
                nc.vector.tensor_mul(
                    gti,
                    s_in[:, DI + q * NI : DI + (q + 1) * NI],
                    s_in[:, EI + q * NI : EI + (q + 1) * NI],
                )
                gtj = per_batch.tile([D, N], BF16, tag="gtj")
                nc.vector.tensor_mul(
                    gtj,
                    s_in[:, DJ + q * N : DJ + (q + 1) * N],
                    s_in[:, EJ + q * N : EJ + (q + 1) * N],
                )
                ps_a = pset_pool.tile([D, NI], F32, tag="ps_a")
                nc.tensor.matmul(ps_a[:], s_wf[:, :D], gti[:])
                ps_c = pset_pool.tile([D, N], F32, tag="ps_c")
                nc.tensor.matmul(ps_c[:], s_wf[:, D:], gtj[:])
                abt = per_batch.tile([D, NI], F32, tag="abt")
                nc.vector.tensor_scalar_add(abt, ps_a[:], s_b1)
                ct = per_batch.tile([D, N], BF16, tag="ct")
                nc.scalar.copy(ct, ps_c[:])

                sig = per_batch.tile([D, SIGW], F32, tag="sig")

                # --- j-block loop ---
                for b in range(NB):
                    j0 = BLK * b
                    L, G, R, W = LS[b], GS[b], RS[b], WS[b]
                    M = 128 // R
                    pss = pss_pool.tile([D, N], F32, tag="pss")
                    for rr in range((R + 1) // 2):
                        nrounds = min(2, R - 2 * rr)
                        psz = psz_pool.tile([D, 2 * N], F32, tag="psz")
                        h2 = h2pool.tile([D, 2 * N], BF16, tag="h2")
                        for rh in range(nrounds):
                            r = 2 * rr + rh
                            hs0 = hpool.tile([D, N], BF16, tag="hs0")
                            hs1 = hpool.tile([D, N], BF16, tag="hs1")
                            for g in range(G):
                                u = r * G + g  # pair index within block
                                for half in range(2):
                                    m = 2 * u + half  # local row 0..7
                                    hs = hs1 if half else hs0
                                    nc.vector.tensor_scalar(
                                        out=hs[:, g * L : (g + 1) * L],
                                        in0=ct[:, j0:N],
                                        scalar1=abt[:, b * RPB + m : b * RPB + m + 1],
                                        scalar2=0.0,
                                        op0=ALU.add,
                                        op1=ALU.max,
                                    )
                            nc.tensor.matmul(
                                psz[0 : D // 2, rh * N : rh * N + W],
                                s_w2,
                                hs0[:, :W],
                                tile_position=(0, 0),
                            )
                            nc.tensor.matmul(
                                psz[D // 2 : D, rh * N : rh * N + W],
                                s_w2,
                                hs1[:, :W],
                                tile_position=(0, 64),
                            )
                        if nrounds == 2:
                            nc.scalar.activation(
                                out=h2[:, : 2 * W].rearrange(
                                    "p (s w) -> p s w", s=2
                                ),
                                in_=psz[:, :].rearrange("p (s w) -> p s w", s=2)[
                                    :, :, :W
                                ],
                                func=AF.Relu,
                                bias=s_b2s,
                            )
                        else:
                            nc.scalar.activation(
                                out=h2[:, :W],
                                in_=psz[:, :W],
                                func=AF.Relu,
                                bias=s_b2s,
                            )
                        for rh in range(nrounds):
                            r = 2 * rr + rh
                            nc.tensor.matmul(
                                pss[M * r : M * (r + 1), :W],
                                s_w3w[:, :M],
                                h2[:, rh * W : rh * W + W],
                                tile_position=(0, M * r),
                            )
                    cb = SIGBASE[b]
                    nc.scalar.activation(
                        out=sig[:, cb : cb + W],
                        in_=pss[:, :W],
                        func=AF.Sigmoid,
                        bias=s_b3,
                    )
                # --- 4 output DMAs for the whole batch (contiguous rows) ---
                for a in range(4):
                    nc.sync.dma_start(
                        out=out[q, a], in_=sig[32 * a : 32 * a + 2, :]
                    )

    nc.compile()
    return nc


def build_in_maps(dom, evo, W1, b1, W2, b2, W3, b3):
    w3w = np.zeros((D, D), np.float32)
    w3w[: D // 2, 0::32] = W3[:, 0:1].repeat(4, axis=1)
    w3w[D // 2 :, 1::32] = W3[:, 0:1].repeat(4, axis=1)
    wb = np.concatenate([W2, w3w], axis=1).astype(bf16_np)
    wf = np.ascontiguousarray(np.concatenate([W1[:D], W1[D:]], axis=1)).astype(bf16_np)
    bbt = np.zeros((D, 3), np.float32)
    bbt[:, 0] = b1
    bbt[:, 1] = np.concatenate([b2, b2])
    bbt[:, 2] = float(b3[0])

    dom_jT = np.concatenate([dom[q].T for q in range(B)], axis=1)
    evo_jT = np.concatenate([evo[q].T for q in range(B)], axis=1)

    in_maps = []
    for k in range(8):
        rows = np.concatenate(
            [BLK * bb_ + RPB * k + np.arange(RPB) for bb_ in range(NB)]
        )
        dom_iT = np.concatenate([dom[q][rows].T for q in range(B)], axis=1)
        evo_iT = np.concatenate([evo[q][rows].T for q in range(B)], axis=1)
        din = np.ascontiguousarray(
            np.concatenate([dom_iT, evo_iT, dom_jT, evo_jT], axis=1)
        ).astype(bf16_np)
        in_maps.append({"din": din, "wf": wf, "bb": bbt, "wb": wb})
    return in_maps


def unpack_results(results):
    S = np.zeros((B, N, N), np.float32)
    for k in range(8):
        o = results[k]["out"]  # [B, 4, 2, SIGW]
        for q in range(B):
            for b in range(NB):
                L, G, R, W = LS[b], GS[b], RS[b], WS[b]
                seg = o[q, :, :, SIGBASE[b] : SIGBASE[b] + W]  # [4, 2, W]
                for a in range(4):
                    for g in range(G):
                        u = (a // (4 // R)) * G + g
                        i = BLK * b + RPB * k + 2 * u
                        S[q, i, BLK * b : BLK * b + L] = seg[a, 0, g * L : (g + 1) * L]
                        S[q, i + 1, BLK * b : BLK * b + L] = seg[
                            a, 1, g * L : (g + 1) * L
                        ]
    upper = np.triu(S, 1)
    return (upper + upper.transpose(0, 2, 1)).astype(np.float32)


def kernel(
    domain_features,
    evolutionary_features,
    W1,
    b1,
    W2,
    b2,
    W3,
    b3,
):
    global LAST_RESULT
    dom = np.ascontiguousarray(np.asarray(domain_features, dtype=np.float32))
    evo = np.ascontiguousarray(np.asarray(evolutionary_features, dtype=np.float32))
    W1 = np.asarray(W1, dtype=np.float32)
    b1 = np.asarray(b1, dtype=np.float32)
    W2 = np.asarray(W2, dtype=np.float32)
    b2 = np.asarray(b2, dtype=np.float32)
    W3 = np.asarray(W3, dtype=np.float32)
    b3 = np.asarray(b3, dtype=np.float32)

    nc = _build()
    in_maps = build_in_maps(dom, evo, W1, b1, W2, b2, W3, b3)

    trace = os.environ.get("KERNEL_TRACE", "0") == "1"
    res = run_bass_kernel_spmd(nc, in_maps, core_ids=list(range(8)), trace=trace)
    LAST_RESULT = res

    return unpack_results(res.results)


# revision 20
# speedup vs baseline: 1.0594x; 1.0367x over previous
"""Trainium2 Bass kernel for nn_CoevolutionAnalyzer (pairwise-MLP coevolution scores).

Math (per batch q):
    g = domain * evo                         [512, 128]
    a = g @ W1[:128], c = g @ W1[128:]       [512, 128]
    h_ij  = relu(a_i + c_j + b1)             [128]
    z2    = W2.T h_ij + b2 ; h2 = relu(z2)   [64]
    s_ij  = sigmoid(W3.h2 + b3)
    out   = triu(s,1) + triu(s,1).T

Sharding (8 cores, one SPMD program):
    Only j >= 64*floor(i/64) is computed (upper triangle padded to the 64-column
    block grid; the pad is discarded on the host via triu). Every core takes 8
    rows of every 64-row block (rows 64*b + 8*k + m) for both batches, so all
    cores run the identical instruction stream; only DMA'd data differs. The
    i-side inputs are host-gathered per core so on-device column indices are
    core-independent.

Per block (8 rows = 4 pairs, j-window [64b, 512) of length L):
    stage1 (DVE/GpSimd): h = relu(cT + a_i + b1) as bf16, two stacked tiles
            (even rows / odd rows), G pairs side by side (G*L <= 512)
    stage2 (PE bf16):   W2.T @ h_even -> psum[0:64], W2.T @ h_odd -> [64:128]
    relu2  (ACT):       h2 = relu(psum + [b2;b2]) -> bf16
    stage3 (PE bf16):   one matmul per round with a replicated-W3 stationary
            [128, 128//R]; pair u's scores land at psum partitions 32u+{0,1}
    sigmoid(ACT):       into a per-batch staging tile [128, 3328]
    out:                2 strided-partition DMAs per batch (rows 32a / 32a+1)
"""

import os

import numpy as np
from ml_dtypes import bfloat16 as bf16_np

import concourse.bass as bass
import concourse.tile as tile
from concourse import bacc, mybir
from concourse.bass_utils import run_bass_kernel_spmd

B = 2
N = 512
D = 128
NB = 8          # number of 64-row j-blocks
BLK = N // NB   # 64
RPB = 8         # rows per core per block
NI = NB * RPB   # i-rows per core per batch (64)
F32 = mybir.dt.float32
F32R = mybir.dt.float32r
BF16 = mybir.dt.bfloat16
AF = mybir.ActivationFunctionType
ALU = mybir.AluOpType

# per-block geometry
LS = [N - BLK * b for b in range(NB)]            # j-window lengths
GS = [min(4, N // L) for L in LS]                # pairs per round
RS = [4 // g for g in GS]                        # rounds per block
WS = [GS[b] * LS[b] for b in range(NB)]          # sig segment widths
SIGW = sum(WS)                                   # 3328
SIGBASE = [sum(WS[:b]) for b in range(NB)]

# how many of the 8 stage-1 rows per block go to GpSimd instead of DVE
GPS_ROWS = 0

LAST_RESULT = None  # set by kernel(); test harness reads exec_time_ns


def _build():
    nc = bacc.Bacc("TRN2", target_bir_lowering=False, debug=False, num_devices=8)

    # din columns: di(2*64) | ei(2*64) | dj(2*512) | ej(2*512)  => 2304 columns
    din = nc.declare_dram_parameter("din", [D, 2 * NI * B + 2 * N * B], BF16, isOutput=False)
    wf = nc.declare_dram_parameter("wf", [D, 2 * D], BF16, isOutput=False)    # w1a|w1b
    bb = nc.declare_dram_parameter("bb", [D, 3], F32, isOutput=False)         # b1|b2s|b3
    wb = nc.declare_dram_parameter("wb", [D, D // 2 + D], BF16, isOutput=False)  # w2|w3w
    out = nc.declare_dram_parameter("out", [B, 4, 2, SIGW], F32, isOutput=True)

    DI, EI, DJ, EJ = 0, NI * B, 2 * NI * B, 2 * NI * B + N * B

    with tile.TileContext(nc) as tc:
        with (
            tc.tile_pool(name="singles", bufs=1) as singles,
            tc.tile_pool(name="per_batch", bufs=2) as per_batch,
            tc.tile_pool(name="hpool", bufs=6) as hpool,
            tc.tile_pool(name="h2pool", bufs=3) as h2pool,
            tc.tile_pool(name="psz", bufs=2, space="PSUM") as psz_pool,
            tc.tile_pool(name="pss", bufs=2, space="PSUM") as pss_pool,
            tc.tile_pool(name="pset", bufs=1, space="PSUM") as pset_pool,
        ):
            s_in = singles.tile([D, 2 * NI * B + 2 * N * B], BF16)
            s_wf = singles.tile([D, 2 * D], BF16)
            s_bb = singles.tile([D, 3], F32)
            s_wb = singles.tile([D, D // 2 + D], BF16)
            # batch-0 j-side + weights first so compute can start early
            nc.scalar.dma_start(out=s_in[:, DJ : DJ + N], in_=din[:, DJ : DJ + N])
            nc.scalar.dma_start(out=s_in[:, EJ : EJ + N], in_=din[:, EJ : EJ + N])
            nc.scalar.dma_start(out=s_wf, in_=wf[:])
            nc.scalar.dma_start(out=s_bb, in_=bb[:])
            nc.scalar.dma_start(out=s_wb, in_=wb[:])
            nc.scalar.dma_start(out=s_in[:, : 2 * NI * B], in_=din[:, : 2 * NI * B])
            nc.sync.dma_start(
                out=s_in[:, DJ + N : DJ + 2 * N], in_=din[:, DJ + N : DJ + 2 * N]
            )
            nc.sync.dma_start(
                out=s_in[:, EJ + N : EJ + 2 * N], in_=din[:, EJ + N : EJ + 2 * N]
            )
            s_w2 = s_wb[:, : D // 2]
            s_w3w = s_wb[:, D // 2 :]
            s_b1 = s_bb[:, 0:1]
            s_b2s = s_bb[:, 1:2]
            s_b3 = s_bb[:, 2:3]

            for q in range(B):
                # --- per-batch setup: gT, aT(+b1), cT ---
                gti = per_batch.tile([D, NI], BF16, tag="gti")
                nc.vector.tensor_mul(
                    gti,
                    s_in[:, DI + q * NI : DI + (q + 1) * NI],
                    s_in[:, EI + q * NI : EI + (q + 1) * NI],
                )
                gtj = per_batch.tile([D, N], BF16, tag="gtj")
                nc.vector.tensor_mul(
                    gtj,
                    s_in[:, DJ + q * N : DJ + (q + 1) * N],
                    s_in[:, EJ + q * N : EJ + (q + 1) * N],
                )
                ps_a = pset_pool.tile([D, NI], F32, tag="ps_a")
                nc.tensor.matmul(ps_a[:], s_wf[:, :D], gti[:])
                ps_c = pset_pool.tile([D, N], F32, tag="ps_c")
                nc.tensor.matmul(ps_c[:], s_wf[:, D:], gtj[:])
                abt = per_batch.tile([D, NI], F32, tag="abt")
                nc.vector.tensor_scalar_add(abt, ps_a[:], s_b1)
                ct = per_batch.tile([D, N], BF16, tag="ct")
                nc.scalar.copy(ct, ps_c[:])

                sig = per_batch.tile([D, SIGW], F32, tag="sig")

                # --- j-block loop ---
                for b in range(NB):
                    j0 = BLK * b
                    L, G, R, W = LS[b], GS[b], RS[b], WS[b]
                    M = 128 // R
                    pss = pss_pool.tile([D, N], F32, tag="pss")
                    for rr in range((R + 1) // 2):
                        nrounds = min(2, R - 2 * rr)
                        psz = psz_pool.tile([D, 2 * N], F32, tag="psz")
                        h2 = h2pool.tile([D, 2 * N], BF16, tag="h2")
                        for rh in range(nrounds):
                            r = 2 * rr + rh
                            hs0 = hpool.tile([D, N], BF16, tag="hs0")
                            hs1 = hpool.tile([D, N], BF16, tag="hs1")
                            for g in range(G):
                                u = r * G + g  # pair index within block
                                for half in range(2):
                                    m = 2 * u + half  # local row 0..7
                                    hs = hs1 if half else hs0
                                    nc.vector.tensor_scalar(
                                        out=hs[:, g * L : (g + 1) * L],
                                        in0=ct[:, j0:N],
                                        scalar1=abt[:, b * RPB + m : b * RPB + m + 1],
                                        scalar2=0.0,
                                        op0=ALU.add,
                                        op1=ALU.max,
                                    )
                            nc.tensor.matmul(
                                psz[0 : D // 2, rh * N : rh * N + W],
                                s_w2,
                                hs0[:, :W],
                                tile_position=(0, 0),
                            )
                            nc.tensor.matmul(
                                psz[D // 2 : D, rh * N : rh * N + W],
                                s_w2,
                                hs1[:, :W],
                                tile_position=(0, 64),
                            )
                        if nrounds == 2:
                            nc.scalar.activation(
                                out=h2[:, : 2 * W].rearrange(
                                    "p (s w) -> p s w", s=2
                                ),
                                in_=psz[:, :].rearrange("p (s w) -> p s w", s=2)[
                                    :, :, :W
                                ],
                                func=AF.Relu,
                                bias=s_b2s,
                            )
                        else:
                            nc.scalar.activation(
                                out=h2[:, :W],
                                in_=psz[:, :W],
                                func=AF.Relu,
                                bias=s_b2s,
                            )
                        for rh in range(nrounds):
                            r = 2 * rr + rh
                            nc.tensor.matmul(
                                pss[M * r : M * (r + 1), :W],
                                s_w3w[:, :M],
                                h2[:, rh * W : rh * W + W],
                                tile_position=(0, M * r),
                            )
                    cb = SIGBASE[b]
                    nc.scalar.activation(
                        out=sig[:, cb : cb + W],
                        in_=pss[:, :W],
                        func=AF.Sigmoid,
                        bias=s_b3,
                    )
                # --- 4 output DMAs for the whole batch (contiguous rows) ---
                for a in range(4):
                    nc.sync.dma_start(
                        out=out[q, a], in_=sig[32 * a : 32 * a + 2, :]
                    )

    nc.compile()
    return nc


def build_in_maps(dom, evo, W1, b1, W2, b2, W3, b3):
    w3w = np.zeros((D, D), np.float32)
    w3w[: D // 2, 0::32] = W3[:, 0:1].repeat(4, axis=1)
    w3w[D // 2 :, 1::32] = W3[:, 0:1].repeat(4, axis=1)
    wb = np.concatenate([W2, w3w], axis=1).astype(bf16_np)
    wf = np.ascontiguousarray(np.concatenate([W1[:D], W1[D:]], axis=1)).astype(bf16_np)
    bbt = np.zeros((D, 3), np.float32)
    bbt[:, 0] = b1
    bbt[:, 1] = np.concatenate([b2, b2])
    bbt[:, 2] = float(b3[0])

    dom_jT = np.concatenate([dom[q].T for q in range(B)], axis=1)
    evo_jT = np.concatenate([evo[q].T for q in range(B)], axis=1)

    in_maps = []
    for k in range(8):
        rows = np.concatenate(
            [BLK * bb_ + RPB * k + np.arange(RPB) for bb_ in range(NB)]
        )
        dom_iT = np.concatenate([dom[q][rows].T for q in range(B)], axis=1)
        evo_iT = np.concatenate([evo[q][rows].T for q in range(B)], axis=1)
        din = np.ascontiguousarray(
            np.concatenate([dom_iT, evo_iT, dom_jT, evo_jT], axis=1)
        ).astype(bf16_np)
        in_maps.append({"din": din, "wf": wf, "bb": bbt, "wb": wb})
    return in_maps


def unpack_results(results):
    S = np.zeros((B, N, N), np.float32)
    for k in range(8):
        o = results[k]["out"]  # [B, 4, 2, SIGW]
        for q in range(B):
            for b in range(NB):
                L, G, R, W = LS[b], GS[b], RS[b], WS[b]
                seg = o[q, :, :, SIGBASE[b] : SIGBASE[b] + W]  # [4, 2, W]
                for a in range(4):
                    for g in range(G):
                        u = (a // (4 // R)) * G + g
                        i = BLK * b + RPB * k + 2 * u
                        S[q, i, BLK * b : BLK * b + L] = seg[a, 0, g * L : (g + 1) * L]
                        S[q, i + 1, BLK * b : BLK * b + L] = seg[
                            a, 1, g * L : (g + 1) * L
                        ]
    upper = np.triu(S, 1)
    return (upper + upper.transpose(0, 2, 1)).astype(np.float32)


def kernel(
    domain_features,
    evolutionary_features,
    W1,
    b1,
    W2,
    b2,
    W3,
    b3,
):
    global LAST_RESULT
    dom = np.ascontiguousarray(np.asarray(domain_features, dtype=np.float32))
    evo = np.ascontiguousarray(np.asarray(evolutionary_features, dtype=np.float32))
    W1 = np.asarray(W1, dtype=np.float32)
    b1 = np.asarray(b1, dtype=np.float32)
    W2 = np.asarray(W2, dtype=np.float32)
    b2 = np.asarray(b2, dtype=np.float32)
    W3 = np.asarray(W3, dtype=np.float32)
    b3 = np.asarray(b3, dtype=np.float32)

    nc = _build()
    in_maps = build_in_maps(dom, evo, W1, b1, W2, b2, W3, b3)

    trace = os.environ.get("KERNEL_TRACE", "0") == "1"
    res = run_bass_kernel_spmd(nc, in_maps, core_ids=list(range(8)), trace=trace)
    LAST_RESULT = res

    return unpack_results(res.results)
